# revision 22
# baseline (speedup 1.0000x reference)
"""Bahdanau attention Trainium2 kernel.

reference:
    dec_proj = decoder_state @ W_s + b_s                        # (B, A)
    enc_proj = einsum("bsh,ha->bsa", encoder_outputs, W_h) + b_h
    scores   = einsum("bsa,a->bs", tanh(dec_proj[:,None,:] + enc_proj), v_a)
    attn     = softmax(scores, axis=1)                          # (B, S)
    context  = einsum("bs,bsh->bh", attn, encoder_outputs)      # (B, H)
    return (context, attn)

Sharding: data-parallel over batch, 4 batches per core on 8 cores, no
collectives. The host passes the encoder shard twice: natural [S, H]
layout and a transposed [H, S] copy (pure layout permutation of the same
fp32 bits — np.swapaxes + ascontiguousarray) so that both contractions
(over h for the projection, over s for the context) can run on the
tensor engine without on-chip transposition of the 64 MB operand.

All large matmuls use float32r (fp32 storage, reduced-precision PE
multiply): at moving free-dim >= 256 it streams 1 column/cycle like
bf16, with ~17x better accuracy (measured 1.3e-4 vs 2.3e-3 max-rel on a
[1024]-contraction).

Per (batch, 512-wide s-block): 64 accumulating matmuls (W_h chunks
stationary) produce enc_proj^T tiles [a(part), s(free)] in PSUM;
tanh+bias is fused on the scalar engine (also evacuating PSUM; bias is
per-partition = dec_proj row + b_s + b_h, computed on-device); the
v_a-weighted reduction over `a` is 8 accumulating M=1 matmuls; softmax
skips the max-subtraction (|score| <= sum|v_a| < 32, so exp cannot
overflow fp32); the exp row is PE-transposed (tiny) so the context
reduction over s runs as accumulating M=1 matmuls against the
natural-layout tiles, normalized at batch end by 1/sum(exp).
"""

import sys

if "/opt/trn_rl_repo" not in sys.path:
    sys.path.insert(0, "/opt/trn_rl_repo")

import numpy as np

import concourse.bass as bass
import concourse.mybir as mybir
import concourse.tile as tile
from concourse.bass_utils import run_bass_kernel_spmd
from concourse.masks import make_identity


def _install_ntff_hook_shim():
    """The agent image's antenv lacks axon_hooks, which run_bass_kernel_spmd
    imports when trace=True. Register a drop-in that drives NRT profiling
    via ctypes against libaxon_pjrt.so. Harmless when tracing is unused."""
    import types

    if "antenv.axon_hooks" in sys.modules:
        return
    try:
        import antenv
    except ImportError:
        return

    import contextlib
    import ctypes
    import os

    mod = types.ModuleType("antenv.axon_hooks")
    mod._HOOK = None
    mod._TRIED = False

    def set_axon_ntff_profile_hook(hook):
        mod._HOOK = hook

    def _via_ctypes(so_path):
        lib = ctypes.CDLL(so_path)
        if not hasattr(lib, "axon_start_nrt_profile"):
            return None
        lib.axon_start_nrt_profile.argtypes = [
            ctypes.POINTER(ctypes.c_int64),
            ctypes.c_size_t,
        ]
        lib.axon_start_nrt_profile.restype = ctypes.c_int64
        lib.axon_stop_nrt_profile.argtypes = [ctypes.c_char_p]
        lib.axon_stop_nrt_profile.restype = ctypes.c_int64

        @contextlib.contextmanager
        def _hook(output_dir, device_ids):
            import jax

            jax.devices()
            if device_ids:
                ids = (ctypes.c_int64 * len(device_ids))(*device_ids)
                rc = lib.axon_start_nrt_profile(ids, len(device_ids))
            else:
                rc = lib.axon_start_nrt_profile(None, 0)
            if rc != 0:
                raise RuntimeError(f"axon_start_nrt_profile rc={rc}")
            try:
                yield
            finally:
                n = lib.axon_stop_nrt_profile(str(output_dir).encode())
                print(f"profile: {n} file(s) -> {output_dir}", file=sys.stderr)

        return _hook

    def get_axon_ntff_profile_hook():
        if mod._HOOK is None and not mod._TRIED:
            mod._TRIED = True
            so = "/opt/axon/libaxon_pjrt.so"
            if os.path.exists(so):
                try:
                    mod._HOOK = _via_ctypes(so)
                except OSError:
                    mod._HOOK = None
        return mod._HOOK

    mod.set_axon_ntff_profile_hook = set_axon_ntff_profile_hook
    mod.get_axon_ntff_profile_hook = get_axon_ntff_profile_hook
    sys.modules["antenv.axon_hooks"] = mod
    antenv.axon_hooks = mod


_install_ntff_hook_shim()


def _enable_ldw_opt():
    """The pinned walrus invocation passes --enable-ldw-opt=false (redundant
    load-weight elision off). The kernel intentionally issues back-to-back
    matmuls with identical stationary operands so this optimization can
    remove half the LDWEIGHTS; flip the flag on for our compiles."""
    from concourse import bass_utils as bu

    if getattr(bu, "_ldw_opt_patched", False):
        return
    orig = bu.bir_verify_and_optimise

    def patched(*args, **kwargs):
        import subprocess

        orig_run = subprocess.run

        def run_patched(cmd, *a, **kw):
            if isinstance(cmd, list):
                cmd = [
                    c.replace("--enable-ldw-opt=false", "--enable-ldw-opt=true")
                    if isinstance(c, str)
                    else c
                    for c in cmd
                ]
            return orig_run(cmd, *a, **kw)

        subprocess.run = run_patched
        try:
            return orig(*args, **kwargs)
        finally:
            subprocess.run = orig_run

    bu.bir_verify_and_optimise = patched
    bu._ldw_opt_patched = True


# _enable_ldw_opt()  # no measurable benefit; keep stock walrus flags


def _split_multi_waits(nc):
    """The TPB ISA has one sync-wait slot per 64B instruction; the pinned
    walrus refuses instructions carrying more (setupSyncWait: 'Too many
    sync wait commands'). Tile's sem assignment can attach several waits
    to one instruction, so spill all but one into standalone poll-sem
    (InstEventSemaphore) instructions on the same engine immediately
    before the instruction — the sequencer executes them in order, which
    is semantically identical (waits AND together)."""
    n_split = 0
    for fn in nc.m.functions:
        for blk in fn.blocks:
            insts = blk.instructions
            new_list = []
            changed = False
            for inst in insts:
                si = inst.sync_info
                waits = list(si.on_wait) if si else []
                if len(waits) > 1 and not isinstance(
                    inst, mybir.InstEventSemaphore
                ):
                    for w in waits[:-1]:
                        ev = mybir.InstEventSemaphore(
                            name=f"I-wsplit-{nc.next_id()}",
                            ins=[],
                            outs=[],
                        )
                        ev.engine = inst.engine
                        ev.sync_info = mybir.SyncInfo(on_wait=[w], on_update=[])
                        nc.register_instruction(ev)
                        new_list.append(ev)
                        n_split += 1
                    inst.sync_info = mybir.SyncInfo(
                        on_wait=[waits[-1]], on_update=list(si.on_update)
                    )
                    changed = True
                new_list.append(inst)
            if changed:
                blk.instructions = new_list
    return n_split


FP32 = mybir.dt.float32
F32R = mybir.dt.float32r
AF = mybir.ActivationFunctionType

P = 128
N_CORES = 8
B_FULL = 32
NB = B_FULL // N_CORES  # batches per core
S = 4096
H = 1024
A = 1024
HC = H // P   # h-chunks (contraction tiles)
AT = A // P   # a-tiles
SBW = 512     # s-block width (one PSUM bank of fp32)


def build_nc(nb=NB, s=S):
    nsb = s // SBW   # s-blocks
    sc_n = SBW // P  # s-chunks per block

    nc = bass.Bass(trn_type="TRN2")

    dec = nc.declare_dram_parameter("dec", [nb, H], FP32, isOutput=False)
    enc = nc.declare_dram_parameter("enc", [nb, s, H], F32R, isOutput=False)
    encT = nc.declare_dram_parameter("encT", [nb, H, s], F32R, isOutput=False)
    w_s = nc.declare_dram_parameter("w_s", [H, A], FP32, isOutput=False)
    w_h = nc.declare_dram_parameter("w_h", [H, A], F32R, isOutput=False)
    b_s = nc.declare_dram_parameter("b_s", [1, A], FP32, isOutput=False)
    b_h = nc.declare_dram_parameter("b_h", [1, A], FP32, isOutput=False)
    v_a = nc.declare_dram_parameter("v_a", [1, A], F32R, isOutput=False)
    ctx_out = nc.declare_dram_parameter("ctx_out", [nb, H], FP32, isOutput=True)
    attn_out = nc.declare_dram_parameter("attn_out", [nb, s], FP32, isOutput=True)

    with tile.TileContext(nc) as tc:
        with (
            tc.tile_pool(name="const", bufs=1) as constp,
            tc.tile_pool(name="setup", bufs=1) as setupp,
            tc.tile_pool(name="encN", bufs=3) as encNp,
            tc.tile_pool(name="encTp", bufs=3) as encTpool,
            tc.tile_pool(name="q", bufs=4) as qp,
            tc.tile_pool(name="small", bufs=2) as smallp,
            tc.tile_pool(name="batch", bufs=2) as batchp,
            tc.tile_pool(name="pq", bufs=4, space="PSUM") as pqp,
            tc.tile_pool(name="psc", bufs=2, space="PSUM") as pscp,
            tc.tile_pool(name="pctx", bufs=1, space="PSUM") as pctxp,
        ):
            ptrp = pscp  # exp-transpose / small psum shares the score banks

            # -------- PE warmup: trip the HAM clock-gate to 2.4 GHz with a
            # dense burst of dummy matmuls before the real work arrives.
            warm = constp.tile([P, P], FP32)
            nc.vector.memset(warm[:], 0.0)
            pwarm = pqp.tile([P, SBW], FP32, tag="pq")
            for i in range(20):
                nc.tensor.matmul(
                    pwarm[:, :P], warm[:], warm[:], skip_group_check=True
                )
            nc.vector.tensor_copy(warm[:, :1], pwarm[:, :1])

            # ---------------- input DMAs, critical-path first ----------------
            # W_h resident [p, hc, a] (float32r) — first big-mm needs it
            whr = constp.tile([P, HC, A], F32R)
            whr_src = w_h.rearrange("(hc p) a -> p hc a", p=P)
            for hc in range(HC):
                eng = nc.sync if hc % 2 == 0 else nc.scalar
                eng.dma_start(whr[:, hc], whr_src[:, hc])

            # W_s on the scalar HWDGE ring; small loads on SWDGE (gpsimd) so
            # the sync ring stays free for the encoder stream.
            ws32 = setupp.tile([P, HC, A], FP32, tag="ws32")
            nc.scalar.dma_start(ws32[:], w_s.rearrange("(hc p) a -> p hc a", p=P))

            bs_row = constp.tile([1, A], FP32)
            nc.gpsimd.dma_start(bs_row[:], b_s[:])
            bh_row = constp.tile([1, A], FP32)
            nc.gpsimd.dma_start(bh_row[:], b_h[:])
            vaT = constp.tile([P, AT], F32R)
            nc.gpsimd.dma_start(vaT[:], v_a[0].rearrange("(t p) -> p t", p=P))
            decn = smallp.tile([nb, H], FP32, tag="decn")
            nc.gpsimd.dma_start(decn[:], dec[:])

            # ---------------- setup ----------------
            ident = constp.tile([P, P], FP32)
            make_identity(nc, ident[:])
            ones_nb = constp.tile([1, nb], FP32)
            nc.vector.memset(ones_nb[:], 1.0)
            ones_p = constp.tile([1, P], FP32)
            nc.vector.memset(ones_p[:], 1.0)
            decT = constp.tile([P, HC, nb], FP32)
            for hc in range(HC):
                ptr_t = ptrp.tile([P, nb], FP32, tag="psc")
                nc.tensor.transpose(
                    ptr_t[:], decn[:, hc * P : (hc + 1) * P], ident[:nb, :nb]
                )
                nc.vector.tensor_copy(decT[:, hc, :], ptr_t[:])

            # bias_tot[:, t, b] = (dec @ W_s)^T + b_s + b_h   [p, t, b]
            bias_tot = constp.tile([P, AT, nb], FP32)
            for t in range(AT):
                pdp = ptrp.tile([P, nb], FP32, tag="psc")
                for hc in range(HC):
                    nc.tensor.matmul(
                        pdp[:],
                        ws32[:, hc, t * P : (t + 1) * P],
                        decT[:, hc, :],
                        start=(hc == 0),
                        stop=False,
                    )
                nc.tensor.matmul(
                    pdp[:],
                    bs_row[:, t * P : (t + 1) * P],
                    ones_nb[:],
                    start=False,
                    stop=False,
                )
                nc.tensor.matmul(
                    pdp[:],
                    bh_row[:, t * P : (t + 1) * P],
                    ones_nb[:],
                    start=False,
                    stop=True,
                )
                nc.vector.tensor_copy(bias_tot[:, t, :], pdp[:])

            # ---------------- main loop ----------------
            for b in range(nb):
                expT32 = batchp.tile([P, nsb * sc_n], FP32, tag="expT32")
                expTr = batchp.tile([P, nsb * sc_n], F32R, tag="expTr")
                sumparts = batchp.tile([1, nsb], FP32, tag="sumparts")
                pctx = pctxp.tile([1, 2, SBW], FP32)

                for sb in range(nsb):
                    # encoder s-block, both layouts, straight from DRAM.
                    # The very first block is split across both HWDGE rings
                    # so the tensor engine can start ~10us earlier.
                    encNt = encNp.tile([P, sc_n, H], F32R)
                    encTt = encTpool.tile([P, HC, SBW], F32R)
                    enc_src = enc[b, sb * SBW : (sb + 1) * SBW, :].rearrange(
                        "(sc p) h -> p sc h", p=P
                    )
                    encT_src = encT[b, :, sb * SBW : (sb + 1) * SBW].rearrange(
                        "(hc p) sw -> p hc sw", p=P
                    )
                    if b == 0 and sb == 0:
                        # strip-level loads so the first accumulation chain
                        # can start as soon as h-chunk 0 lands
                        for hc in range(HC):
                            eng = nc.sync if hc % 2 == 0 else nc.scalar
                            eng.dma_start(encTt[:, hc], encT_src[:, hc])
                        ss = sc_n // 2
                        nc.sync.dma_start(encNt[:, :ss], enc_src[:, :ss])
                        nc.scalar.dma_start(encNt[:, ss:], enc_src[:, ss:])
                    else:
                        nc.sync.dma_start(encNt[:], enc_src)
                        nc.sync.dma_start(encTt[:], encT_src)

                    # big matmul + tanh + v_a reduction
                    psc = pscp.tile([1, SBW], FP32, tag="psc")
                    for t in range(AT):
                        pq = pqp.tile([P, SBW], FP32, tag="pq")
                        for hc in range(HC):
                            nc.tensor.matmul(
                                pq[:],
                                whr[:, hc, t * P : (t + 1) * P],
                                encTt[:, hc, :],
                                start=(hc == 0),
                                stop=(hc == HC - 1),
                            )
                        qt = qp.tile([P, SBW], F32R, tag="qt")
                        nc.scalar.activation(
                            qt[:], pq[:], AF.Tanh, bias=bias_tot[:, t, b : b + 1]
                        )
                        nc.tensor.matmul(
                            psc[:],
                            vaT[:, t : t + 1],
                            qt[:],
                            start=(t == 0),
                            stop=(t == AT - 1),
                            skip_group_check=True,
                        )

                    # exp (no max subtraction needed: |score| < 32)
                    exp_sb = smallp.tile([1, SBW], FP32, tag="exp_sb")
                    nc.scalar.activation(
                        exp_sb[:],
                        psc[:],
                        AF.Exp,
                        accum_out=sumparts[:, sb : sb + 1],
                    )

                    # transpose exp row into partition dim
                    ptr_t = ptrp.tile([P, sc_n], FP32, tag="psc")
                    for sc in range(sc_n):
                        nc.tensor.transpose(
                            ptr_t[:, sc : sc + 1],
                            exp_sb[:, sc * P : (sc + 1) * P],
                            ident[:1, :1],
                        )
                    nc.vector.tensor_copy(
                        expT32[:, sb * sc_n : (sb + 1) * sc_n], ptr_t[:]
                    )
                    nc.vector.tensor_copy(
                        expTr[:, sb * sc_n : (sb + 1) * sc_n], ptr_t[:]
                    )

                    # context accumulation: ctx_un[h] += exp[s] * enc[s, h]
                    for sc in range(sc_n):
                        for blk in range(2):
                            nc.tensor.matmul(
                                pctx[:, blk, :],
                                expTr[:, sb * sc_n + sc : sb * sc_n + sc + 1],
                                encNt[:, sc, blk * SBW : (blk + 1) * SBW],
                                start=(sb == 0 and sc == 0),
                                stop=(sb == nsb - 1 and sc == sc_n - 1),
                                skip_group_check=True,
                            )

                # -------- batch tail: normalize --------
                sumexp = smallp.tile([1, 1], FP32, tag="sumexp")
                nc.vector.tensor_reduce(
                    sumexp[:],
                    sumparts[:],
                    axis=mybir.AxisListType.X,
                    op=mybir.AluOpType.add,
                )
                inv = smallp.tile([1, 1], FP32, tag="inv")
                nc.vector.reciprocal(inv[:], sumexp[:])

                # broadcast inv across partitions via K=1 matmul
                pinv = ptrp.tile([P, 1], FP32, tag="psc")
                nc.tensor.matmul(pinv[:], ones_p[:], inv[:])
                inv128 = smallp.tile([P, 1], FP32, tag="inv128")
                nc.vector.tensor_copy(inv128[:], pinv[:])

                attnT = smallp.tile([P, nsb * sc_n], FP32, tag="attnT")
                nc.vector.tensor_scalar_mul(attnT[:], expT32[:], inv128[:])
                nc.sync.dma_start(
                    attn_out[b].rearrange("(j p) -> p j", p=P), attnT[:]
                )

                ctxrow = smallp.tile([1, H], FP32, tag="ctxrow")
                nc.vector.tensor_scalar_mul(
                    ctxrow[:], pctx[:].rearrange("p x y -> p (x y)"), inv[:]
                )
                nc.sync.dma_start(ctx_out[b : b + 1, :], ctxrow[:])

    _split_multi_waits(nc)
    return nc


_NC_CACHE = {}


def _get_nc():
    key = (NB, S)
    if key not in _NC_CACHE:
        _NC_CACHE[key] = build_nc()
    return _NC_CACHE[key]


def kernel(decoder_state, encoder_outputs, W_s, b_s, W_h, b_h, v_a):
    decoder_state = np.ascontiguousarray(np.asarray(decoder_state, dtype=np.float32))
    encoder_outputs = np.ascontiguousarray(
        np.asarray(encoder_outputs, dtype=np.float32)
    )
    W_s = np.ascontiguousarray(np.asarray(W_s, dtype=np.float32))
    W_h = np.ascontiguousarray(np.asarray(W_h, dtype=np.float32))
    b_s = np.asarray(b_s, dtype=np.float32).reshape(1, A)
    b_h = np.asarray(b_h, dtype=np.float32).reshape(1, A)
    v_a = np.asarray(v_a, dtype=np.float32).reshape(1, A)

    nc = _get_nc()
    core_ids = list(range(N_CORES))
    in_maps = []
    for i in core_ids:
        lo, hi = i * NB, (i + 1) * NB
        enc_i = np.ascontiguousarray(encoder_outputs[lo:hi])
        encT_i = np.ascontiguousarray(enc_i.swapaxes(1, 2))
        in_maps.append(
            {
                "dec": decoder_state[lo:hi],
                "enc": enc_i,
                "encT": encT_i,
                "w_s": W_s,
                "w_h": W_h,
                "b_s": b_s,
                "b_h": b_h,
                "v_a": v_a,
            }
        )

    res = run_bass_kernel_spmd(nc, in_maps, core_ids)
    outs = res.results
    context = np.concatenate([outs[i]["ctx_out"] for i in core_ids], axis=0)
    attn = np.concatenate([outs[i]["attn_out"] for i in core_ids], axis=0)
    return (context, attn)


# revision 24
# speedup vs baseline: 1.0402x; 1.0402x over previous
"""Bahdanau attention Trainium2 kernel.

reference:
    dec_proj = decoder_state @ W_s + b_s                        # (B, A)
    enc_proj = einsum("bsh,ha->bsa", encoder_outputs, W_h) + b_h
    scores   = einsum("bsa,a->bs", tanh(dec_proj[:,None,:] + enc_proj), v_a)
    attn     = softmax(scores, axis=1)                          # (B, S)
    context  = einsum("bs,bsh->bh", attn, encoder_outputs)      # (B, H)
    return (context, attn)

Sharding: data-parallel over batch, 4 batches per core on 8 cores, no
collectives. The host passes the encoder shard twice: natural [S, H]
layout and a transposed [H, S] copy (pure layout permutation of the same
fp32 bits — np.swapaxes + ascontiguousarray) so that both contractions
(over h for the projection, over s for the context) can run on the
tensor engine without on-chip transposition of the 64 MB operand.

All large matmuls use float32r (fp32 storage, reduced-precision PE
multiply): at moving free-dim >= 256 it streams 1 column/cycle like
bf16, with ~17x better accuracy (measured 1.3e-4 vs 2.3e-3 max-rel on a
[1024]-contraction).

Per (batch, 512-wide s-block): 64 accumulating matmuls (W_h chunks
stationary) produce enc_proj^T tiles [a(part), s(free)] in PSUM;
tanh+bias is fused on the scalar engine (also evacuating PSUM; bias is
per-partition = dec_proj row + b_s + b_h, computed on-device); the
v_a-weighted reduction over `a` is 8 accumulating M=1 matmuls; softmax
skips the max-subtraction (|score| <= sum|v_a| < 32, so exp cannot
overflow fp32); the exp row is PE-transposed (tiny) so the context
reduction over s runs as accumulating M=1 matmuls against the
natural-layout tiles, normalized at batch end by 1/sum(exp).
"""

import sys

if "/opt/trn_rl_repo" not in sys.path:
    sys.path.insert(0, "/opt/trn_rl_repo")

import numpy as np

import concourse.bass as bass
import concourse.mybir as mybir
import concourse.tile as tile
from concourse.bass_utils import run_bass_kernel_spmd
from concourse.masks import make_identity


def _install_ntff_hook_shim():
    """The agent image's antenv lacks axon_hooks, which run_bass_kernel_spmd
    imports when trace=True. Register a drop-in that drives NRT profiling
    via ctypes against libaxon_pjrt.so. Harmless when tracing is unused."""
    import types

    if "antenv.axon_hooks" in sys.modules:
        return
    try:
        import antenv
    except ImportError:
        return

    import contextlib
    import ctypes
    import os

    mod = types.ModuleType("antenv.axon_hooks")
    mod._HOOK = None
    mod._TRIED = False

    def set_axon_ntff_profile_hook(hook):
        mod._HOOK = hook

    def _via_ctypes(so_path):
        lib = ctypes.CDLL(so_path)
        if not hasattr(lib, "axon_start_nrt_profile"):
            return None
        lib.axon_start_nrt_profile.argtypes = [
            ctypes.POINTER(ctypes.c_int64),
            ctypes.c_size_t,
        ]
        lib.axon_start_nrt_profile.restype = ctypes.c_int64
        lib.axon_stop_nrt_profile.argtypes = [ctypes.c_char_p]
        lib.axon_stop_nrt_profile.restype = ctypes.c_int64

        @contextlib.contextmanager
        def _hook(output_dir, device_ids):
            import jax

            jax.devices()
            if device_ids:
                ids = (ctypes.c_int64 * len(device_ids))(*device_ids)
                rc = lib.axon_start_nrt_profile(ids, len(device_ids))
            else:
                rc = lib.axon_start_nrt_profile(None, 0)
            if rc != 0:
                raise RuntimeError(f"axon_start_nrt_profile rc={rc}")
            try:
                yield
            finally:
                n = lib.axon_stop_nrt_profile(str(output_dir).encode())
                print(f"profile: {n} file(s) -> {output_dir}", file=sys.stderr)

        return _hook

    def get_axon_ntff_profile_hook():
        if mod._HOOK is None and not mod._TRIED:
            mod._TRIED = True
            so = "/opt/axon/libaxon_pjrt.so"
            if os.path.exists(so):
                try:
                    mod._HOOK = _via_ctypes(so)
                except OSError:
                    mod._HOOK = None
        return mod._HOOK

    mod.set_axon_ntff_profile_hook = set_axon_ntff_profile_hook
    mod.get_axon_ntff_profile_hook = get_axon_ntff_profile_hook
    sys.modules["antenv.axon_hooks"] = mod
    antenv.axon_hooks = mod


_install_ntff_hook_shim()


def _enable_ldw_opt():
    """The pinned walrus invocation passes --enable-ldw-opt=false (redundant
    load-weight elision off). The kernel intentionally issues back-to-back
    matmuls with identical stationary operands so this optimization can
    remove half the LDWEIGHTS; flip the flag on for our compiles."""
    from concourse import bass_utils as bu

    if getattr(bu, "_ldw_opt_patched", False):
        return
    orig = bu.bir_verify_and_optimise

    def patched(*args, **kwargs):
        import subprocess

        orig_run = subprocess.run

        def run_patched(cmd, *a, **kw):
            if isinstance(cmd, list):
                cmd = [
                    c.replace("--enable-ldw-opt=false", "--enable-ldw-opt=true")
                    if isinstance(c, str)
                    else c
                    for c in cmd
                ]
            return orig_run(cmd, *a, **kw)

        subprocess.run = run_patched
        try:
            return orig(*args, **kwargs)
        finally:
            subprocess.run = orig_run

    bu.bir_verify_and_optimise = patched
    bu._ldw_opt_patched = True


# _enable_ldw_opt()  # no measurable benefit; keep stock walrus flags


def _split_multi_waits(nc):
    """The TPB ISA has one sync-wait slot per 64B instruction; the pinned
    walrus refuses instructions carrying more (setupSyncWait: 'Too many
    sync wait commands'). Tile's sem assignment can attach several waits
    to one instruction, so spill all but one into standalone poll-sem
    (InstEventSemaphore) instructions on the same engine immediately
    before the instruction — the sequencer executes them in order, which
    is semantically identical (waits AND together)."""
    n_split = 0
    for fn in nc.m.functions:
        for blk in fn.blocks:
            insts = blk.instructions
            new_list = []
            changed = False
            for inst in insts:
                si = inst.sync_info
                waits = list(si.on_wait) if si else []
                if len(waits) > 1 and not isinstance(
                    inst, mybir.InstEventSemaphore
                ):
                    for w in waits[:-1]:
                        ev = mybir.InstEventSemaphore(
                            name=f"I-wsplit-{nc.next_id()}",
                            ins=[],
                            outs=[],
                        )
                        ev.engine = inst.engine
                        ev.sync_info = mybir.SyncInfo(on_wait=[w], on_update=[])
                        nc.register_instruction(ev)
                        new_list.append(ev)
                        n_split += 1
                    inst.sync_info = mybir.SyncInfo(
                        on_wait=[waits[-1]], on_update=list(si.on_update)
                    )
                    changed = True
                new_list.append(inst)
            if changed:
                blk.instructions = new_list
    return n_split


FP32 = mybir.dt.float32
F32R = mybir.dt.float32r
AF = mybir.ActivationFunctionType

P = 128
N_CORES = 8
B_FULL = 32
NB = B_FULL // N_CORES  # batches per core
S = 4096
H = 1024
A = 1024
HC = H // P   # h-chunks (contraction tiles)
AT = A // P   # a-tiles
SBW = 512     # s-block width (one PSUM bank of fp32)


def build_nc(nb=NB, s=S):
    nsb = s // SBW   # s-blocks
    sc_n = SBW // P  # s-chunks per block

    nc = bass.Bass(trn_type="TRN2")

    dec = nc.declare_dram_parameter("dec", [nb, H], FP32, isOutput=False)
    enc = nc.declare_dram_parameter("enc", [nb, s, H], F32R, isOutput=False)
    encT = nc.declare_dram_parameter("encT", [nb, H, s], F32R, isOutput=False)
    w_s = nc.declare_dram_parameter("w_s", [H, A], FP32, isOutput=False)
    w_h = nc.declare_dram_parameter("w_h", [H, A], F32R, isOutput=False)
    b_s = nc.declare_dram_parameter("b_s", [1, A], FP32, isOutput=False)
    b_h = nc.declare_dram_parameter("b_h", [1, A], FP32, isOutput=False)
    v_a = nc.declare_dram_parameter("v_a", [1, A], F32R, isOutput=False)
    ctx_out = nc.declare_dram_parameter("ctx_out", [nb, H], FP32, isOutput=True)
    attn_out = nc.declare_dram_parameter("attn_out", [nb, s], FP32, isOutput=True)

    with tile.TileContext(nc) as tc:
        with (
            tc.tile_pool(name="const", bufs=1) as constp,
            tc.tile_pool(name="setup", bufs=1) as setupp,
            tc.tile_pool(name="encN", bufs=3) as encNp,
            tc.tile_pool(name="encTp", bufs=3) as encTpool,
            tc.tile_pool(name="q", bufs=4) as qp,
            tc.tile_pool(name="small", bufs=2) as smallp,
            tc.tile_pool(name="batch", bufs=2) as batchp,
            tc.tile_pool(name="pq", bufs=4, space="PSUM") as pqp,
            tc.tile_pool(name="psc", bufs=2, space="PSUM") as pscp,
            tc.tile_pool(name="pctx", bufs=1, space="PSUM") as pctxp,
        ):
            ptrp = pscp  # exp-transpose / small psum shares the score banks

            # -------- PE warmup: trip the HAM clock-gate to 2.4 GHz with a
            # dense burst of dummy matmuls before the real work arrives.
            warm = constp.tile([P, P], FP32)
            nc.vector.memset(warm[:], 0.0)
            pwarm = pqp.tile([P, SBW], FP32, tag="pq")
            for i in range(20):
                nc.tensor.matmul(
                    pwarm[:, :P], warm[:], warm[:], skip_group_check=True
                )
            nc.vector.tensor_copy(warm[:, :1], pwarm[:, :1])

            # ---------------- input DMAs, critical-path first ----------------
            # W_h resident [p, hc, a] (float32r) — first big-mm needs it
            whr = constp.tile([P, HC, A], F32R)
            whr_src = w_h.rearrange("(hc p) a -> p hc a", p=P)
            nc.sync.dma_start(whr[:, : HC // 2], whr_src[:, : HC // 2])
            nc.scalar.dma_start(whr[:, HC // 2 :], whr_src[:, HC // 2 :])

            # W_s on the scalar HWDGE ring; small loads on SWDGE (gpsimd) so
            # the sync ring stays free for the encoder stream.
            ws32 = setupp.tile([P, HC, A], FP32, tag="ws32")
            nc.scalar.dma_start(ws32[:], w_s.rearrange("(hc p) a -> p hc a", p=P))

            bs_row = constp.tile([1, A], FP32)
            nc.gpsimd.dma_start(bs_row[:], b_s[:])
            bh_row = constp.tile([1, A], FP32)
            nc.gpsimd.dma_start(bh_row[:], b_h[:])
            vaT = constp.tile([P, AT], F32R)
            nc.gpsimd.dma_start(vaT[:], v_a[0].rearrange("(t p) -> p t", p=P))
            decn = smallp.tile([nb, H], FP32, tag="decn")
            nc.gpsimd.dma_start(decn[:], dec[:])

            # ---------------- setup ----------------
            ident = constp.tile([P, P], FP32)
            make_identity(nc, ident[:])
            ones_nb = constp.tile([1, nb], FP32)
            nc.vector.memset(ones_nb[:], 1.0)
            ones_p = constp.tile([1, P], FP32)
            nc.vector.memset(ones_p[:], 1.0)
            decT = constp.tile([P, HC, nb], FP32)
            for hc in range(HC):
                ptr_t = ptrp.tile([P, nb], FP32, tag="psc")
                nc.tensor.transpose(
                    ptr_t[:], decn[:, hc * P : (hc + 1) * P], ident[:nb, :nb]
                )
                nc.vector.tensor_copy(decT[:, hc, :], ptr_t[:])

            # bias_tot[:, t, b] = (dec @ W_s)^T + b_s + b_h   [p, t, b]
            bias_tot = constp.tile([P, AT, nb], FP32)
            for t in range(AT):
                pdp = ptrp.tile([P, nb], FP32, tag="psc")
                for hc in range(HC):
                    nc.tensor.matmul(
                        pdp[:],
                        ws32[:, hc, t * P : (t + 1) * P],
                        decT[:, hc, :],
                        start=(hc == 0),
                        stop=False,
                    )
                nc.tensor.matmul(
                    pdp[:],
                    bs_row[:, t * P : (t + 1) * P],
                    ones_nb[:],
                    start=False,
                    stop=False,
                )
                nc.tensor.matmul(
                    pdp[:],
                    bh_row[:, t * P : (t + 1) * P],
                    ones_nb[:],
                    start=False,
                    stop=True,
                )
                nc.vector.tensor_copy(bias_tot[:, t, :], pdp[:])

            # ---------------- main loop ----------------
            for b in range(nb):
                expT32 = batchp.tile([P, nsb * sc_n], FP32, tag="expT32")
                expTr = batchp.tile([P, nsb * sc_n], F32R, tag="expTr")
                sumparts = batchp.tile([1, nsb], FP32, tag="sumparts")
                pctx = pctxp.tile([1, 2, SBW], FP32)

                for sb in range(nsb):
                    # encoder s-block, both layouts, straight from DRAM.
                    # The very first block is split across both HWDGE rings
                    # so the tensor engine can start ~10us earlier.
                    encNt = encNp.tile([P, sc_n, H], F32R)
                    encTt = encTpool.tile([P, HC, SBW], F32R)
                    enc_src = enc[b, sb * SBW : (sb + 1) * SBW, :].rearrange(
                        "(sc p) h -> p sc h", p=P
                    )
                    encT_src = encT[b, :, sb * SBW : (sb + 1) * SBW].rearrange(
                        "(hc p) sw -> p hc sw", p=P
                    )
                    if b == 0 and sb == 0:
                        hh = HC // 2
                        nc.sync.dma_start(encTt[:, :hh], encT_src[:, :hh])
                        nc.scalar.dma_start(encTt[:, hh:], encT_src[:, hh:])
                        ss = sc_n // 2
                        nc.sync.dma_start(encNt[:, :ss], enc_src[:, :ss])
                        nc.scalar.dma_start(encNt[:, ss:], enc_src[:, ss:])
                    else:
                        nc.sync.dma_start(encNt[:], enc_src)
                        nc.sync.dma_start(encTt[:], encT_src)

                    # big matmul + tanh + v_a reduction
                    psc = pscp.tile([1, SBW], FP32, tag="psc")
                    for t in range(AT):
                        pq = pqp.tile([P, SBW], FP32, tag="pq")
                        for hc in range(HC):
                            nc.tensor.matmul(
                                pq[:],
                                whr[:, hc, t * P : (t + 1) * P],
                                encTt[:, hc, :],
                                start=(hc == 0),
                                stop=(hc == HC - 1),
                            )
                        qt = qp.tile([P, SBW], F32R, tag="qt")
                        nc.scalar.activation(
                            qt[:], pq[:], AF.Tanh, bias=bias_tot[:, t, b : b + 1]
                        )
                        nc.tensor.matmul(
                            psc[:],
                            vaT[:, t : t + 1],
                            qt[:],
                            start=(t == 0),
                            stop=(t == AT - 1),
                            skip_group_check=True,
                        )

                    # exp (no max subtraction needed: |score| < 32)
                    exp_sb = smallp.tile([1, SBW], FP32, tag="exp_sb")
                    nc.scalar.activation(
                        exp_sb[:],
                        psc[:],
                        AF.Exp,
                        accum_out=sumparts[:, sb : sb + 1],
                    )

                    # transpose exp row into partition dim
                    ptr_t = ptrp.tile([P, sc_n], FP32, tag="psc")
                    for sc in range(sc_n):
                        nc.tensor.transpose(
                            ptr_t[:, sc : sc + 1],
                            exp_sb[:, sc * P : (sc + 1) * P],
                            ident[:1, :1],
                        )
                    nc.vector.tensor_copy(
                        expT32[:, sb * sc_n : (sb + 1) * sc_n], ptr_t[:]
                    )
                    nc.vector.tensor_copy(
                        expTr[:, sb * sc_n : (sb + 1) * sc_n], ptr_t[:]
                    )

                    # context accumulation: ctx_un[h] += exp[s] * enc[s, h]
                    for sc in range(sc_n):
                        for blk in range(2):
                            nc.tensor.matmul(
                                pctx[:, blk, :],
                                expTr[:, sb * sc_n + sc : sb * sc_n + sc + 1],
                                encNt[:, sc, blk * SBW : (blk + 1) * SBW],
                                start=(sb == 0 and sc == 0),
                                stop=(sb == nsb - 1 and sc == sc_n - 1),
                                skip_group_check=True,
                            )

                # -------- batch tail: normalize --------
                sumexp = smallp.tile([1, 1], FP32, tag="sumexp")
                nc.vector.tensor_reduce(
                    sumexp[:],
                    sumparts[:],
                    axis=mybir.AxisListType.X,
                    op=mybir.AluOpType.add,
                )
                inv = smallp.tile([1, 1], FP32, tag="inv")
                nc.vector.reciprocal(inv[:], sumexp[:])

                # broadcast inv across partitions via K=1 matmul
                pinv = ptrp.tile([P, 1], FP32, tag="psc")
                nc.tensor.matmul(pinv[:], ones_p[:], inv[:])
                inv128 = smallp.tile([P, 1], FP32, tag="inv128")
                nc.vector.tensor_copy(inv128[:], pinv[:])

                attnT = smallp.tile([P, nsb * sc_n], FP32, tag="attnT")
                nc.vector.tensor_scalar_mul(attnT[:], expT32[:], inv128[:])
                nc.sync.dma_start(
                    attn_out[b].rearrange("(j p) -> p j", p=P), attnT[:]
                )

                ctxrow = smallp.tile([1, H], FP32, tag="ctxrow")
                nc.vector.tensor_scalar_mul(
                    ctxrow[:], pctx[:].rearrange("p x y -> p (x y)"), inv[:]
                )
                nc.sync.dma_start(ctx_out[b : b + 1, :], ctxrow[:])

    _split_multi_waits(nc)
    return nc


_NC_CACHE = {}


def _get_nc():
    key = (NB, S)
    if key not in _NC_CACHE:
        _NC_CACHE[key] = build_nc()
    return _NC_CACHE[key]


def kernel(decoder_state, encoder_outputs, W_s, b_s, W_h, b_h, v_a):
    decoder_state = np.ascontiguousarray(np.asarray(decoder_state, dtype=np.float32))
    encoder_outputs = np.ascontiguousarray(
        np.asarray(encoder_outputs, dtype=np.float32)
    )
    W_s = np.ascontiguousarray(np.asarray(W_s, dtype=np.float32))
    W_h = np.ascontiguousarray(np.asarray(W_h, dtype=np.float32))
    b_s = np.asarray(b_s, dtype=np.float32).reshape(1, A)
    b_h = np.asarray(b_h, dtype=np.float32).reshape(1, A)
    v_a = np.asarray(v_a, dtype=np.float32).reshape(1, A)

    nc = _get_nc()
    core_ids = list(range(N_CORES))
    in_maps = []
    for i in core_ids:
        lo, hi = i * NB, (i + 1) * NB
        enc_i = np.ascontiguousarray(encoder_outputs[lo:hi])
        encT_i = np.ascontiguousarray(enc_i.swapaxes(1, 2))
        in_maps.append(
            {
                "dec": decoder_state[lo:hi],
                "enc": enc_i,
                "encT": encT_i,
                "w_s": W_s,
                "w_h": W_h,
                "b_s": b_s,
                "b_h": b_h,
                "v_a": v_a,
            }
        )

    res = run_bass_kernel_spmd(nc, in_maps, core_ids)
    outs = res.results
    context = np.concatenate([outs[i]["ctx_out"] for i in core_ids], axis=0)
    attn = np.concatenate([outs[i]["attn_out"] for i in core_ids], axis=0)
    return (context, attn)
